# revision 7
# baseline (speedup 1.0000x reference)
"""Trainium2 Bass kernel for ConformalGQA, v3 (fp8 split-DoubleRow).

Math identical to reference modulo softmax shift invariance: the -0.5|q|^2
term in the scores is constant over the softmax (key) axis, so it is dropped
entirely. Scores become (q.k - 0.5|k|^2)/8, bounded above by |q|^2/16 ~ 6, so
exp never overflows fp32 and needs no max pass. The -0.5|k|^2/8 term rides
the per-partition bias of the Exp activation.

Sharding: 8-way tensor-parallel over heads (core c: Q heads 4c..4c+3, KV
head c). Each core emits a full (4096, 2048) bf16 partial; host sums.

v3: the Q/K/V projections and the out-projection run as fp8e4 DoubleRow
matmuls (0.5 cyc/row, two 128-contraction planes per instruction = 4x bf16
throughput) with 3-term residual splits for bf16-level accuracy:
  A@B ~= Ah@Bh + Ar@Bh + Ah@Br,  Xh = fp8(X*s), Xr = fp8(X*s - Xh).
fp8's wide exponent range makes the residuals directly representable at the
same scale, so terms need no rescaling and DR-pairs freely. Attention S / PV
stay bf16 (1-term fp8 there fails the 2e-2 gate; split costs more than it
saves). All scales are powers of two, folded for free into the rope tables
(1/(sx*sw)), the PV ones-column (sx*sw/sy), and the output eviction scale
(1/(sy*swo)).

Per core, per batch (t=1024):
 - xT hi/res fp8 chunks DMA'd; Wq/Wk/Wv hi/res column shards projected with
   weights stationary into PSUM f32 chunks [128, 512] via 24 DR matmuls.
 - RoPE: PSUM chunk evicted to SBUF f32 (Act), cos-mul + signed-sin
   shifted-muls (shift = +-32 partitions; muls on GPSIMD, add on DVE),
   emitted as bf16 qhat/khat (true scale; tables carry the dequant).
   khat duplicated to partitions 64:128 so both heads of a pair run
   S-matmuls via tile_position (0,0)/(64,0).
 - S^T computed per (head, kc) into [128, <=512] PSUM chunks with k on
   partitions; the diagonal block is zeroed post-exp by a triangular mask.
 - P^T = Exp(S^T/8 + bias) -> bf16, bias = -0.0625|k|^2.
 - PV: yhat[65, q] += [V|c].T @ P^T accumulated over kc into two
   single-bank [65, 512] PSUM halves; row 64 is the softmax denominator
   times c = sx*sw/sy so the normalized quotient lands at y*sy.
 - normalize: DVE reciprocal row, GPSIMD partition_broadcast, mul into a
   f32 staging half, then split-evicted as fp8 hi/res into ytnh/ytnr
   [128, 2, T] (dim1 = head-pair, the DR contraction pairing for outproj).
 - out proj: 3-term DR (ytn hi/res stationary x Wo hi/res moving), PSUM
   chunks evicted bf16 with scale 1/(sy*swo), DMA'd out per row block.
 - Whole thing software-pipelined across batches as in v2.
"""

import sys

for _p in ("/opt/trn_rl_repo",):
    if _p not in sys.path:
        sys.path.insert(0, _p)

import numpy as np
import ml_dtypes
from contextlib import ExitStack

import concourse.bass as bass
import concourse.mybir as mybir
import concourse.tile as tile
from concourse import bacc
from concourse.bass_utils import run_bass_kernel_spmd

F32R = mybir.dt.float32r
F32 = mybir.dt.float32
BF16 = mybir.dt.bfloat16
FP8 = mybir.dt.float8e4
AF = mybir.ActivationFunctionType
BF = ml_dtypes.bfloat16
F8 = ml_dtypes.float8_e4m3
DR = mybir.MatmulPerfMode.DoubleRow

B, T, D, KV = 4, 1024, 2048, 512
H, HKV, HD = 32, 8, 64
P = 128
NCORES = 8
HPC = H // NCORES          # 4 q heads per core
DOUT = HPC * HD            # 256 q-proj cols per core
NDC = D // P               # 16 contraction chunks
NPAIR = NDC // 2           # 8 DR contraction pairs
NTC = T // P               # 8 token chunks per batch
ROPE_BASE = 10000.0

SX = 8.0                   # x scale
SW = 512.0                 # Wq/Wk/Wv scale (shared; folded into rope tables)
SWO = 512.0                # Wo scale
SY = 32.0                  # ytn scale
ONES_C = SX * SW / SY      # 128.0: PV denominator column value
OUT_SC = 1.0 / (SY * SWO)  # output eviction scale

_COMPILED = {}


def _chunks_for(kc):
    """Natural S/PV q-chunks for key block kc (bf16: any width is full
    rate). Chunks never straddle the 512 boundary (PSUM half split)."""
    q0 = kc * P
    out = []
    c0 = q0
    while c0 < T:
        c1 = min(T, 512 if c0 < 512 else T)
        out.append((c0, c1))
        c0 = c1
    return q0, out


def _build_nc():
    nc = bacc.Bacc("TRN2", target_bir_lowering=False, debug=False,
                   num_devices=NCORES)

    xh = nc.dram_tensor("xh", [D, B * T], FP8, kind="ExternalInput")
    xr = nc.dram_tensor("xr", [D, B * T], FP8, kind="ExternalInput")
    wqh = nc.dram_tensor("wqh", [P, NDC, DOUT], FP8, kind="ExternalInput")
    wqr = nc.dram_tensor("wqr", [P, NDC, DOUT], FP8, kind="ExternalInput")
    wkvh = nc.dram_tensor("wkvh", [P, NDC, 2 * HD], FP8, kind="ExternalInput")
    wkvr = nc.dram_tensor("wkvr", [P, NDC, 2 * HD], FP8, kind="ExternalInput")
    woh = nc.dram_tensor("woh", [P, 2, D], FP8, kind="ExternalInput")
    wor = nc.dram_tensor("wor", [P, 2, D], FP8, kind="ExternalInput")
    cc = nc.dram_tensor("cc", [P, T], F32, kind="ExternalInput")
    ss = nc.dram_tensor("ss", [P, T], F32, kind="ExternalInput")
    tri2 = nc.dram_tensor("tri2", [P, 2, P], BF16, kind="ExternalInput")
    o64 = nc.dram_tensor("o64", [64, 2], F32R, kind="ExternalInput")
    idb = nc.dram_tensor("idb", [P, 64], F32, kind="ExternalInput")
    out = nc.dram_tensor("out", [B * T, D], BF16, kind="ExternalOutput")

    with tile.TileContext(nc) as tc:
        with ExitStack() as ctx:
            cpool = ctx.enter_context(tc.tile_pool(name="consts", bufs=1))
            wpool = ctx.enter_context(tc.tile_pool(name="weights", bufs=1))
            xpool = ctx.enter_context(tc.tile_pool(name="x", bufs=8))
            spool = ctx.enter_context(tc.tile_pool(name="stage", bufs=4))
            qpool = ctx.enter_context(tc.tile_pool(name="qk", bufs=2))
            vpool = ctx.enter_context(tc.tile_pool(name="v", bufs=2))
            fpool = ctx.enter_context(tc.tile_pool(name="pt", bufs=3))
            npool = ctx.enter_context(tc.tile_pool(name="norm", bufs=4))
            ypool = ctx.enter_context(tc.tile_pool(name="ytn", bufs=2))
            opool = ctx.enter_context(tc.tile_pool(name="ostg", bufs=3))
            psy = ctx.enter_context(tc.tile_pool(name="psy", bufs=2, space="PSUM"))
            pss = ctx.enter_context(tc.tile_pool(name="pss", bufs=4, space="PSUM"))
            psm = ctx.enter_context(tc.tile_pool(name="psm", bufs=2, space="PSUM"))

            # ---- early consts (needed by first projections/rope) ----
            t_wkvh = wpool.tile([P, NDC, 2 * HD], FP8)
            nc.sync.dma_start(t_wkvh[:], wkvh.ap())
            t_wkvr = wpool.tile([P, NDC, 2 * HD], FP8)
            nc.sync.dma_start(t_wkvr[:], wkvr.ap())
            t_wqh = wpool.tile([P, NDC, DOUT], FP8)
            t_wqr = wpool.tile([P, NDC, DOUT], FP8)
            t_cc = cpool.tile([P, T], F32)
            t_ss = cpool.tile([P, T], F32)

            xh3 = xh.ap().rearrange("(c p) t -> p c t", p=P)  # [128, 16, 4096]
            xr3 = xr.ap().rearrange("(c p) t -> p c t", p=P)

            def late_consts():
                t_tri2 = cpool.tile([P, 2, P], BF16)
                nc.sync.dma_start(t_tri2[:], tri2.ap())
                t_o64 = cpool.tile([64, 2], F32R)
                nc.sync.dma_start(t_o64[:], o64.ap())
                t_idb = cpool.tile([P, 64], F32)
                nc.sync.dma_start(t_idb[:], idb.ap())
                t_woh = wpool.tile([P, 2, D], FP8)
                nc.sync.dma_start(t_woh[:], woh.ap())
                t_wor = wpool.tile([P, 2, D], FP8)
                nc.sync.dma_start(t_wor[:], wor.ap())
                return t_tri2, t_o64, t_woh, t_wor, t_idb

            lc = None

            def rope_half(pj, dst, rows, half, sign_dup, sb=None):
                """Evict PSUM proj chunk, rope it, write bf16 into dst."""
                c0 = half * 512
                if sb is None:
                    sb = spool.tile([P, 512], F32, tag="qsb")
                    nc.any.tensor_copy(sb[0:rows, :], pj[0:rows, :])
                t1 = spool.tile([P, 512], F32, tag="t1")
                nc.any.tensor_mul(
                    t1[0:rows, :], sb[0:rows, :], t_cc[0:rows, c0:c0 + 512])
                t2 = spool.tile([P, 512], F32, tag="t2")
                for bp2 in range(0, rows, 64):
                    nc.gpsimd.tensor_mul(
                        t2[bp2:bp2 + 32, :], sb[bp2 + 32:bp2 + 64, :],
                        t_ss[bp2 + 32:bp2 + 64, c0:c0 + 512])
                    nc.gpsimd.tensor_mul(
                        t2[bp2 + 32:bp2 + 64, :], sb[bp2:bp2 + 32, :],
                        t_ss[bp2:bp2 + 32, c0:c0 + 512])
                nc.any.tensor_add(
                    dst[0:rows, c0:c0 + 512], t1[0:rows, :], t2[0:rows, :])
                if sign_dup:
                    nc.any.tensor_copy(
                        dst[64:128, c0:c0 + 512], dst[0:64, c0:c0 + 512])

            def proj_rope_stage(b):
                """Load xT for batch b, project Q/K/V (3-term fp8 DR), rope."""
                nonlocal lc
                tok0 = b * T
                xts_h, xts_r = [], []
                for qtr in range(4):
                    xt_h = xpool.tile([P, 4, T], FP8, tag="xth",
                                      name=f"xth_{b}_{qtr}")
                    xt_r = xpool.tile([P, 4, T], FP8, tag="xtr",
                                      name=f"xtr_{b}_{qtr}")
                    if b == 0:
                        # fine-grained loads so batch-0 projections start
                        # as soon as the first contraction chunks land;
                        # wq arrives in pair halves, rope tables last
                        for i in range(4):
                            nc.sync.dma_start(
                                xt_h[:, i, :],
                                xh3[:, qtr * 4 + i, tok0:tok0 + T])
                            if qtr == 0 and i == 0:
                                nc.sync.dma_start(
                                    t_wqh[:, :, 0:P], wqh.ap()[:, :, 0:P])
                        nc.sync.dma_start(
                            xt_r[:], xr3[:, qtr * 4:(qtr + 1) * 4,
                                         tok0:tok0 + T])
                        if qtr == 0:
                            nc.sync.dma_start(
                                t_wqh[:, :, P:2 * P], wqh.ap()[:, :, P:2 * P])
                        if qtr == 1:
                            nc.sync.dma_start(t_wqr[:], wqr.ap())
                        if qtr == 3:
                            nc.sync.dma_start(t_cc[:], cc.ap())
                            nc.sync.dma_start(t_ss[:], ss.ap())
                    else:
                        nc.sync.dma_start(
                            xt_h[:], xh3[:, qtr * 4:(qtr + 1) * 4,
                                         tok0:tok0 + T])
                        nc.sync.dma_start(
                            xt_r[:], xr3[:, qtr * 4:(qtr + 1) * 4,
                                         tok0:tok0 + T])
                    xts_h.append(xt_h)
                    xts_r.append(xt_r)
                if b == 0:
                    lc = late_consts()

                def xsrc(xts, pi, cols):
                    """Moving AP for DR pair pi: [128, 2, len(cols)]."""
                    q, r = (2 * pi) // 4, (2 * pi) % 4
                    return xts[q][:, r:r + 2, cols[0]:cols[1]]

                qh = [qpool.tile([P, T], BF16, tag="qh", bufs=4,
                                 name=f"qh_{b}_{i}") for i in range(2)]
                kh = qpool.tile([P, T], BF16, tag="kh", name=f"kh_{b}")
                k2 = qpool.tile([64, T], F32R, tag="k2", name=f"k2_{b}")
                vstage = []

                # interleave kv and q-pair0 chunks so both pj slots
                # stream against arriving xT chunks; then q-pair1.
                def kv_chunk(half):
                    pj = psm.tile([P, 512], F32, tag="pj",
                                  name=f"kvpj_{b}_{half}")
                    cols = (half * 512, (half + 1) * 512)
                    n = 0
                    for wt, xt in ((t_wkvh, xts_h), (t_wkvr, xts_h),
                                   (t_wkvh, xts_r)):
                        for pi in range(NPAIR):
                            nc.tensor.matmul(
                                pj[:], wt[:, 2 * pi:2 * pi + 2, :],
                                xsrc(xt, pi, cols),
                                start=(n == 0), stop=(n == 3 * NPAIR - 1),
                                perf_mode=DR)
                            n += 1
                    # single eviction: rows 0:64 = K dims (rope input),
                    # rows 64:128 = V dims (read later by the transposes)
                    kvsb = spool.tile([P, 512], F32, tag="kvsb", bufs=4,
                                      name=f"kvsb_{b}_{half}")
                    nc.any.tensor_copy(kvsb[:], pj[:])
                    vstage.append(kvsb)
                    rope_half(pj, kh, 64, half, sign_dup=True, sb=kvsb)

                def q_chunk(pairi, half):
                    pj = psm.tile([P, 512], F32, tag="pj",
                                  name=f"qpj_{b}_{pairi}_{half}")
                    cols = (half * 512, (half + 1) * 512)
                    n = 0
                    for wt, xt in ((t_wqh, xts_h), (t_wqr, xts_h),
                                   (t_wqh, xts_r)):
                        for pi in range(NPAIR):
                            nc.tensor.matmul(
                                pj[:],
                                wt[:, 2 * pi:2 * pi + 2,
                                   pairi * P:(pairi + 1) * P],
                                xsrc(xt, pi, cols),
                                start=(n == 0), stop=(n == 3 * NPAIR - 1),
                                perf_mode=DR)
                            n += 1
                    rope_half(pj, qh[pairi], 128, half, sign_dup=False)

                kv_chunk(0)
                q_chunk(0, 0)
                kv_chunk(1)
                q_chunk(0, 1)
                q_chunk(1, 0)
                q_chunk(1, 1)

                # |k|^2 -> per-partition bias  (transposed via PE)
                t_o64, t_idb = lc[1], lc[4]
                nc.scalar.activation(k2[:], kh[0:64, :], AF.Square)
                nsq = psm.tile([P, 512], F32, tag="pj", name=f"nsq_{b}")
                for kc in range(NTC):
                    nc.tensor.matmul(
                        nsq[:, 2 * kc:2 * kc + 2], k2[:, kc * P:(kc + 1) * P],
                        t_o64[:], start=True, stop=True)
                kb = qpool.tile([P, NTC], F32, tag="kb", name=f"kb_{b}")
                nc.vector.tensor_scalar_mul(
                    kb[:],
                    nsq[:, 0:2 * NTC]
                    .rearrange("p (c two) -> p c two", two=2)[:, :, 0],
                    -0.0625)

                # V transposed into [token, hd | c] layout via PE transpose;
                # the ones-column carries c = sx*sw/sy for free dequant
                vh = vpool.tile([P, NTC, HD + 1], BF16, tag="vh",
                                name=f"vh_{b}")
                nc.vector.memset(vh[:, :, HD:HD + 1], ONES_C)
                for tcn in range(NTC):
                    tp = pss.tile([P, 64], F32, tag="stp", name=f"tp_{b}_{tcn}")
                    vsrc = vstage[tcn // 4]
                    nc.tensor.transpose(
                        tp[:], vsrc[64:128, (tcn % 4) * P:(tcn % 4 + 1) * P],
                        t_idb[64:128, :])
                    nc.any.tensor_copy(vh[:, tcn, 0:HD], tp[:])
                return dict(b=b, qh=qh, kh=kh, kb=kb, vh=vh)

            def attn_stage(st):
                b, qh, kh, kb, vh = st["b"], st["qh"], st["kh"], st["kb"], st["vh"]
                t_tri2 = lc[0]
                ytnh = ypool.tile([P, 2, T], FP8, tag="ytnh", bufs=3,
                                  name=f"ytnh_{b}")
                ytnr = ypool.tile([P, 2, T], FP8, tag="ytnr", bufs=3,
                                  name=f"ytnr_{b}")

                def normalize_half(yh_half, hq, pairi, bp, tag):
                    """One half of softmax-normalize as soon as its PV
                    contributions are complete; frees the yh slot early.
                    Emits fp8 hi/res planes of y*sy."""
                    rsb = npool.tile([1, 512], F32R, tag="rsb")
                    with nc.allow_low_precision(reason="recip row"):
                        nc.vector.reciprocal(rsb[:], yh_half[64:65, :])
                    rbc = npool.tile([64, 512], F32R, tag="rbc")
                    nc.gpsimd.partition_broadcast(rbc[:], rsb[:])
                    tmpf = npool.tile([P, 512], F32, tag="ytmp", bufs=2)
                    tmp = tmpf[bp:bp + 64, :]
                    nc.any.tensor_mul(tmp, yh_half[0:64, :], rbc[:])
                    dsth = ytnh[bp:bp + 64, pairi, hq * 512:(hq + 1) * 512]
                    nc.any.tensor_copy(dsth, tmp)
                    nc.any.tensor_sub(
                        ytnr[bp:bp + 64, pairi, hq * 512:(hq + 1) * 512],
                        tmp, dsth)

                for h in range(HPC):
                    pairi, bp = h // 2, 64 * (h % 2)
                    yhA = psy.tile([65, 512], F32, tag="yh",
                                   name=f"yhA_{b}_{h}")
                    yhB = psy.tile([65, 512], F32, tag="yh",
                                   name=f"yhB_{b}_{h}")
                    for kc in range(NTC):
                        q0 = kc * P
                        _, chs = _chunks_for(kc)
                        pt = fpool.tile([P, T], BF16, tag="pt", bufs=6)
                        for (c0, c1) in chs:
                            stp = pss.tile([P, 512], F32, tag="stp")
                            nc.tensor.matmul(
                                stp[:, 0:c1 - c0],
                                kh[bp:bp + 64, kc * P:(kc + 1) * P],
                                qh[pairi][bp:bp + 64, c0:c1],
                                start=True, stop=True,
                                tile_position=(bp, 0))
                            nc.scalar.activation(
                                pt[:, c0:c1], stp[:, 0:c1 - c0], AF.Exp,
                                bias=kb[:, kc:kc + 1], scale=0.125)
                        # causal mask on the diagonal block
                        nc.any.tensor_mul(
                            pt[:, q0:q0 + P], pt[:, q0:q0 + P],
                            t_tri2[:, 0, :])
                        # split PV at the diagonal boundary so the
                        # unmasked bulk starts right after exp (no mask hop)
                        pvs = []
                        for (c0, c1) in chs:
                            if c0 == q0 and c1 > q0 + P:
                                pvs += [(c0, q0 + P), (q0 + P, c1)]
                            else:
                                pvs.append((c0, c1))
                        for (c0, c1) in pvs:
                            half = yhA if c0 < 512 else yhB
                            off = 0 if c0 < 512 else 512
                            nc.tensor.matmul(
                                half[:, c0 - off:c1 - off], vh[:, kc, :],
                                pt[:, c0:c1],
                                start=(kc == 0),
                                stop=(kc == (3 if half is yhA else NTC - 1)),
                                skip_group_check=True)
                        if kc == 3:
                            normalize_half(yhA, 0, pairi, bp, "A")
                    normalize_half(yhB, 1, pairi, bp, "B")

                return ytnh, ytnr

            def out_stage(b, ytn):
                ytnh, ytnr = ytn
                tok0 = b * T
                t_woh, t_wor = lc[2], lc[3]
                # ---------- output projection: 3-term split DR ----------
                for tcn in range(NTC):
                    ostg = opool.tile([P, D], BF16, tag="ostg", bufs=2)
                    for oc in range(4):
                        ops_ = pss.tile([P, 512], F32, tag="stp",
                                        name=f"ops_{b}_{tcn}_{oc}")
                        terms = ((ytnh, t_woh), (ytnr, t_woh), (ytnh, t_wor))
                        for n, (yt, wt) in enumerate(terms):
                            nc.tensor.matmul(
                                ops_[:],
                                yt[:, :, tcn * P:(tcn + 1) * P],
                                wt[:, :, oc * 512:(oc + 1) * 512],
                                start=(n == 0), stop=(n == 2),
                                perf_mode=DR)
                        nc.any.tensor_scalar_mul(
                            ostg[:, oc * 512:(oc + 1) * 512], ops_[:], OUT_SC)
                    if b == B - 1 and tcn >= NTC - 2:
                        # pipeline the final drain: two half-width DMAs so
                        # the first launches while oc2/oc3 still evict
                        for hh in range(2):
                            nc.sync.dma_start(
                                out.ap()[tok0 + tcn * P: tok0 + (tcn + 1) * P,
                                         hh * 1024:(hh + 1) * 1024],
                                ostg[:, hh * 1024:(hh + 1) * 1024])
                    else:
                        nc.sync.dma_start(
                            out.ap()[tok0 + tcn * P: tok0 + (tcn + 1) * P, :],
                            ostg[:])

            # software pipeline with deferred out-projections: proj/rope
            # of b+1 runs ahead of attention of b, and outproj of b is
            # issued a full stage later so it stays available as PE filler
            # for the Act-rate-limited late attention phases.
            # Order: P0 P1 A0 P2 A1 O0 P3 A2 O1 A3 O2 O3
            st0 = proj_rope_stage(0)
            st1 = proj_rope_stage(1)
            y0 = attn_stage(st0)
            st2 = proj_rope_stage(2)
            y1 = attn_stage(st1)
            out_stage(0, y0)
            st3 = proj_rope_stage(3)
            y2 = attn_stage(st2)
            out_stage(1, y1)
            y3 = attn_stage(st3)
            out_stage(2, y2)
            out_stage(3, y3)

    nc.finalize()
    return nc


def _host_consts():
    inv = 1.0 / (ROPE_BASE ** (np.arange(0, HD, 2, dtype=np.float32) / HD))
    ang = np.arange(T, dtype=np.float32)[:, None] * inv[None, :]  # [T, 32]
    dq = 1.0 / (SX * SW)
    cosr = (np.cos(ang) * dq).T.astype(np.float32)                 # [32, T]
    sinr = (np.sin(ang) * dq).T.astype(np.float32)
    cc = np.tile(cosr, (4, 1))                                     # [128, T]
    ss = np.tile(np.concatenate([sinr, -sinr], axis=0), (2, 1))
    tri = np.triu(np.ones((P, P), np.float32))
    tri2 = np.stack([tri, tri], axis=1)                            # [128,2,128]
    return {
        "cc": np.ascontiguousarray(cc),
        "ss": np.ascontiguousarray(ss),
        "tri2": np.ascontiguousarray(tri2.astype(BF)),
        "o64": np.ones((64, 2), np.float32),
        "idb": np.concatenate([np.zeros((64, 64), np.float32),
                               np.eye(64, dtype=np.float32)], axis=0),
    }


def _split8(a, scale):
    """a*scale ~= hi + res, both fp8 bytes."""
    hi = np.clip(a * scale, -240.0, 240.0).astype(F8)
    res = (a * scale - hi.astype(np.float32)).astype(F8)
    return hi, res


def kernel(x, Wq, Wk, Wv, Wo):
    x = np.asarray(x, np.float32)
    Wq = np.asarray(Wq, np.float32)
    Wk = np.asarray(Wk, np.float32)
    Wv = np.asarray(Wv, np.float32)
    Wo = np.asarray(Wo, np.float32)
    b, t, d = x.shape

    key = "nc"
    if key not in _COMPILED:
        _COMPILED[key] = _build_nc()
    nc = _COMPILED[key]

    xT = x.reshape(b * t, d).T                                   # [2048, 4096]
    xh_np, xr_np = _split8(np.ascontiguousarray(xT), SX)
    consts = _host_consts()

    in_maps = []
    for c in range(NCORES):
        wq_c = (Wq[:, c * DOUT:(c + 1) * DOUT] * SW).reshape(NDC, P, DOUT) \
            .transpose(1, 0, 2)
        wqh_c = np.clip(wq_c, -240, 240).astype(F8)
        wqr_c = (wq_c - wqh_c.astype(np.float32)).astype(F8)
        wkv_np = np.concatenate(
            [Wk[:, c * HD:(c + 1) * HD], Wv[:, c * HD:(c + 1) * HD]],
            axis=1) * SW
        wkv_c = wkv_np.reshape(NDC, P, 2 * HD).transpose(1, 0, 2)
        wkvh_c = np.clip(wkv_c, -240, 240).astype(F8)
        wkvr_c = (wkv_c - wkvh_c.astype(np.float32)).astype(F8)
        wo_c = (Wo[c * DOUT:(c + 1) * DOUT, :] * SWO).reshape(2, P, d) \
            .transpose(1, 0, 2)
        woh_c = np.clip(wo_c, -240, 240).astype(F8)
        wor_c = (wo_c - woh_c.astype(np.float32)).astype(F8)
        m = {"xh": xh_np, "xr": xr_np,
             "wqh": np.ascontiguousarray(wqh_c),
             "wqr": np.ascontiguousarray(wqr_c),
             "wkvh": np.ascontiguousarray(wkvh_c),
             "wkvr": np.ascontiguousarray(wkvr_c),
             "woh": np.ascontiguousarray(woh_c),
             "wor": np.ascontiguousarray(wor_c)}
        m.update(consts)
        in_maps.append(m)

    res = run_bass_kernel_spmd(nc, in_maps, list(range(NCORES)))
    acc = res.results[0]["out"].astype(np.float32)
    for c in range(1, NCORES):
        acc = acc + res.results[c]["out"].astype(np.float32)
    return acc.reshape(b, t, d)


if __name__ == "__main__":
    rng = np.random.default_rng(0)
    x = rng.standard_normal((B, T, D), dtype=np.float32)
    Wq = (rng.standard_normal((D, D), dtype=np.float32) * 0.02)
    Wk = (rng.standard_normal((D, KV), dtype=np.float32) * 0.02)
    Wv = (rng.standard_normal((D, KV), dtype=np.float32) * 0.02)
    Wo = (rng.standard_normal((D, D), dtype=np.float32) * 0.02)
    y = kernel(x=x, Wq=Wq, Wk=Wk, Wv=Wv, Wo=Wo)
    print("out", y.shape, y.dtype, np.abs(y).max())


# revision 25
# speedup vs baseline: 1.0199x; 1.0199x over previous
"""Trainium2 Bass kernel for ConformalGQA, v3 (fp8 split-DoubleRow).

Math identical to reference modulo softmax shift invariance: the -0.5|q|^2
term in the scores is constant over the softmax (key) axis, so it is dropped
entirely. Scores become (q.k - 0.5|k|^2)/8, bounded above by |q|^2/16 ~ 6, so
exp never overflows fp32 and needs no max pass. The -0.5|k|^2/8 term rides
the per-partition bias of the Exp activation.

Sharding: 8-way tensor-parallel over heads (core c: Q heads 4c..4c+3, KV
head c). Each core emits a full (4096, 2048) bf16 partial; host sums.

v3: the Q/K/V projections and the out-projection run as fp8e4 DoubleRow
matmuls (0.5 cyc/row, two 128-contraction planes per instruction = 4x bf16
throughput) with 3-term residual splits for bf16-level accuracy:
  A@B ~= Ah@Bh + Ar@Bh + Ah@Br,  Xh = fp8(X*s), Xr = fp8(X*s - Xh).
fp8's wide exponent range makes the residuals directly representable at the
same scale, so terms need no rescaling and DR-pairs freely. Attention S / PV
stay bf16 (1-term fp8 there fails the 2e-2 gate; split costs more than it
saves). All scales are powers of two, folded for free into the rope tables
(1/(sx*sw)), the PV ones-column (sx*sw/sy), and the output eviction scale
(1/(sy*swo)).

Per core, per batch (t=1024):
 - xT hi/res fp8 chunks DMA'd; Wq/Wk/Wv hi/res column shards projected with
   weights stationary into PSUM f32 chunks [128, 512] via 24 DR matmuls.
 - RoPE: PSUM chunk evicted to SBUF f32 (Act), cos-mul + signed-sin
   shifted-muls (shift = +-32 partitions; muls on GPSIMD, add on DVE),
   emitted as bf16 qhat/khat (true scale; tables carry the dequant).
   khat duplicated to partitions 64:128 so both heads of a pair run
   S-matmuls via tile_position (0,0)/(64,0).
 - S^T computed per (head, kc) into [128, <=512] PSUM chunks with k on
   partitions; the diagonal block is zeroed post-exp by a triangular mask.
 - P^T = Exp(S^T/8 + bias) -> bf16, bias = -0.0625|k|^2.
 - PV: yhat[65, q] += [V|c].T @ P^T accumulated over kc into two
   single-bank [65, 512] PSUM halves; row 64 is the softmax denominator
   times c = sx*sw/sy so the normalized quotient lands at y*sy.
 - normalize: DVE reciprocal row, GPSIMD partition_broadcast, mul into a
   f32 staging half, then split-evicted as fp8 hi/res into ytnh/ytnr
   [128, 2, T] (dim1 = head-pair, the DR contraction pairing for outproj).
 - out proj: 3-term DR (ytn hi/res stationary x Wo hi/res moving), PSUM
   chunks evicted bf16 with scale 1/(sy*swo), DMA'd out per row block.
 - Whole thing software-pipelined across batches as in v2.
"""

import sys

for _p in ("/opt/trn_rl_repo",):
    if _p not in sys.path:
        sys.path.insert(0, _p)

import numpy as np
import ml_dtypes
from contextlib import ExitStack

import concourse.bass as bass
import concourse.mybir as mybir
import concourse.tile as tile
from concourse import bacc
from concourse.bass_utils import run_bass_kernel_spmd

F32R = mybir.dt.float32r
F32 = mybir.dt.float32
BF16 = mybir.dt.bfloat16
FP8 = mybir.dt.float8e4
AF = mybir.ActivationFunctionType
BF = ml_dtypes.bfloat16
F8 = ml_dtypes.float8_e4m3
DR = mybir.MatmulPerfMode.DoubleRow

B, T, D, KV = 4, 1024, 2048, 512
H, HKV, HD = 32, 8, 64
P = 128
NCORES = 8
HPC = H // NCORES          # 4 q heads per core
DOUT = HPC * HD            # 256 q-proj cols per core
NDC = D // P               # 16 contraction chunks
NPAIR = NDC // 2           # 8 DR contraction pairs
NTC = T // P               # 8 token chunks per batch
ROPE_BASE = 10000.0

SX = 8.0                   # x scale
SW = 512.0                 # Wq/Wk/Wv scale (shared; folded into rope tables)
SWO = 512.0                # Wo scale
SY = 32.0                  # ytn scale
ONES_C = SX * SW / SY      # 128.0: PV denominator column value
OUT_SC = 1.0 / (SY * SWO)  # output eviction scale

_COMPILED = {}


def _chunks_for(kc):
    """Natural S/PV q-chunks for key block kc (bf16: any width is full
    rate). Chunks never straddle the 512 boundary (PSUM half split)."""
    q0 = kc * P
    out = []
    c0 = q0
    while c0 < T:
        c1 = min(T, 512 if c0 < 512 else T)
        out.append((c0, c1))
        c0 = c1
    return q0, out


def _build_nc():
    nc = bacc.Bacc("TRN2", target_bir_lowering=False, debug=False,
                   num_devices=NCORES)

    xh = nc.dram_tensor("xh", [D, B * T], FP8, kind="ExternalInput")
    xr = nc.dram_tensor("xr", [D, B * T], FP8, kind="ExternalInput")
    wqh = nc.dram_tensor("wqh", [P, NDC, DOUT], FP8, kind="ExternalInput")
    wqr = nc.dram_tensor("wqr", [P, NDC, DOUT], FP8, kind="ExternalInput")
    wkvh = nc.dram_tensor("wkvh", [P, NDC, 2 * HD], FP8, kind="ExternalInput")
    wkvr = nc.dram_tensor("wkvr", [P, NDC, 2 * HD], FP8, kind="ExternalInput")
    woh = nc.dram_tensor("woh", [P, 2, D], FP8, kind="ExternalInput")
    wor = nc.dram_tensor("wor", [P, 2, D], FP8, kind="ExternalInput")
    cc = nc.dram_tensor("cc", [P, T], BF16, kind="ExternalInput")
    ss = nc.dram_tensor("ss", [P, T], BF16, kind="ExternalInput")
    tri2 = nc.dram_tensor("tri2", [P, 2, P], BF16, kind="ExternalInput")
    o64 = nc.dram_tensor("o64", [64, 2], F32R, kind="ExternalInput")
    idb = nc.dram_tensor("idb", [P, 64], BF16, kind="ExternalInput")
    out = nc.dram_tensor("out", [B * T, D], BF16, kind="ExternalOutput")

    with tile.TileContext(nc) as tc:
        with ExitStack() as ctx:
            cpool = ctx.enter_context(tc.tile_pool(name="consts", bufs=1))
            wpool = ctx.enter_context(tc.tile_pool(name="weights", bufs=1))
            xpool = ctx.enter_context(tc.tile_pool(name="x", bufs=8))
            spool = ctx.enter_context(tc.tile_pool(name="stage", bufs=4))
            qpool = ctx.enter_context(tc.tile_pool(name="qk", bufs=2))
            vpool = ctx.enter_context(tc.tile_pool(name="v", bufs=2))
            fpool = ctx.enter_context(tc.tile_pool(name="pt", bufs=3))
            npool = ctx.enter_context(tc.tile_pool(name="norm", bufs=4))
            ypool = ctx.enter_context(tc.tile_pool(name="ytn", bufs=2))
            opool = ctx.enter_context(tc.tile_pool(name="ostg", bufs=3))
            psy = ctx.enter_context(tc.tile_pool(name="psy", bufs=2, space="PSUM"))
            pss = ctx.enter_context(tc.tile_pool(name="pss", bufs=4, space="PSUM"))
            psm = ctx.enter_context(tc.tile_pool(name="psm", bufs=2, space="PSUM"))

            # ---- early consts (needed by first projections/rope) ----
            t_wkvh = wpool.tile([P, NDC, 2 * HD], FP8)
            nc.sync.dma_start(t_wkvh[:], wkvh.ap())
            t_wkvr = wpool.tile([P, NDC, 2 * HD], FP8)
            nc.sync.dma_start(t_wkvr[:], wkvr.ap())
            t_wqh = wpool.tile([P, NDC, DOUT], FP8)
            t_wqr = wpool.tile([P, NDC, DOUT], FP8)
            t_cc = cpool.tile([P, T], BF16)
            t_ss = cpool.tile([P, T], BF16)

            xh3 = xh.ap().rearrange("(c p) t -> p c t", p=P)  # [128, 16, 4096]
            xr3 = xr.ap().rearrange("(c p) t -> p c t", p=P)

            def late_consts():
                t_tri2 = cpool.tile([P, 2, P], BF16)
                nc.sync.dma_start(t_tri2[:], tri2.ap())
                t_o64 = cpool.tile([64, 2], F32R)
                nc.sync.dma_start(t_o64[:], o64.ap())
                t_idb = cpool.tile([P, 64], BF16)
                nc.sync.dma_start(t_idb[:], idb.ap())
                t_woh = wpool.tile([P, 2, D], FP8)
                nc.sync.dma_start(t_woh[:], woh.ap())
                t_wor = wpool.tile([P, 2, D], FP8)
                nc.sync.dma_start(t_wor[:], wor.ap())
                return t_tri2, t_o64, t_woh, t_wor, t_idb

            lc = None

            def rope_half(pj, dst, rows, half, sign_dup, sb=None):
                """Evict PSUM proj chunk, rope it, write bf16 into dst.
                All staging is bf16 so DVE 2x modes apply."""
                c0 = half * 512
                if sb is None:
                    sb = spool.tile([P, 512], BF16, tag="qsb")
                    nc.vector.tensor_copy(sb[0:rows, :], pj[0:rows, :])
                t1 = spool.tile([P, 512], BF16, tag="t1")
                nc.vector.tensor_mul(
                    t1[0:rows, :], sb[0:rows, :], t_cc[0:rows, c0:c0 + 512])
                t2 = spool.tile([P, 512], BF16, tag="t2")
                for bp2 in range(0, rows, 64):
                    nc.gpsimd.tensor_mul(
                        t2[bp2:bp2 + 32, :], sb[bp2 + 32:bp2 + 64, :],
                        t_ss[bp2 + 32:bp2 + 64, c0:c0 + 512])
                    nc.gpsimd.tensor_mul(
                        t2[bp2 + 32:bp2 + 64, :], sb[bp2:bp2 + 32, :],
                        t_ss[bp2:bp2 + 32, c0:c0 + 512])
                nc.vector.tensor_add(
                    dst[0:rows, c0:c0 + 512], t1[0:rows, :], t2[0:rows, :])
                if sign_dup:
                    nc.gpsimd.tensor_copy(
                        dst[64:128, c0:c0 + 512], dst[0:64, c0:c0 + 512])

            def proj_rope_stage(b):
                """Load xT for batch b, project Q/K/V (3-term fp8 DR), rope."""
                nonlocal lc
                tok0 = b * T
                xts_h, xts_r = [], []
                for qtr in range(4):
                    xt_h = xpool.tile([P, 4, T], FP8, tag="xth",
                                      name=f"xth_{b}_{qtr}")
                    xt_r = xpool.tile([P, 4, T], FP8, tag="xtr",
                                      name=f"xtr_{b}_{qtr}")
                    if b == 0:
                        # fine-grained loads so batch-0 projections start
                        # as soon as the first contraction chunks land;
                        # wq arrives in pair halves, rope tables last
                        for i in range(4):
                            nc.sync.dma_start(
                                xt_h[:, i, :],
                                xh3[:, qtr * 4 + i, tok0:tok0 + T])
                            if qtr == 0 and i == 0:
                                nc.sync.dma_start(
                                    t_wqh[:, :, 0:P], wqh.ap()[:, :, 0:P])
                        nc.sync.dma_start(
                            xt_r[:], xr3[:, qtr * 4:(qtr + 1) * 4,
                                         tok0:tok0 + T])
                        if qtr == 0:
                            nc.sync.dma_start(
                                t_wqh[:, :, P:2 * P], wqh.ap()[:, :, P:2 * P])
                        if qtr == 1:
                            nc.sync.dma_start(t_wqr[:], wqr.ap())
                        if qtr == 3:
                            nc.sync.dma_start(t_cc[:], cc.ap())
                            nc.sync.dma_start(t_ss[:], ss.ap())
                    else:
                        nc.sync.dma_start(
                            xt_h[:], xh3[:, qtr * 4:(qtr + 1) * 4,
                                         tok0:tok0 + T])
                        nc.sync.dma_start(
                            xt_r[:], xr3[:, qtr * 4:(qtr + 1) * 4,
                                         tok0:tok0 + T])
                    xts_h.append(xt_h)
                    xts_r.append(xt_r)
                if b == 0:
                    lc = late_consts()

                def xsrc(xts, pi, cols):
                    """Moving AP for DR pair pi: [128, 2, len(cols)]."""
                    q, r = (2 * pi) // 4, (2 * pi) % 4
                    return xts[q][:, r:r + 2, cols[0]:cols[1]]

                qh = [qpool.tile([P, T], BF16, tag="qh", bufs=4,
                                 name=f"qh_{b}_{i}") for i in range(2)]
                kh = qpool.tile([P, T], BF16, tag="kh", name=f"kh_{b}")
                k2 = qpool.tile([64, T], F32R, tag="k2", name=f"k2_{b}")
                vstage = []

                # interleave kv and q-pair0 chunks so both pj slots
                # stream against arriving xT chunks; then q-pair1.
                def kv_chunk(half):
                    pj = psm.tile([P, 512], F32, tag="pj",
                                  name=f"kvpj_{b}_{half}")
                    cols = (half * 512, (half + 1) * 512)
                    n = 0
                    for wt, xt in ((t_wkvh, xts_h), (t_wkvr, xts_h),
                                   (t_wkvh, xts_r)):
                        for pi in range(NPAIR):
                            nc.tensor.matmul(
                                pj[:], wt[:, 2 * pi:2 * pi + 2, :],
                                xsrc(xt, pi, cols),
                                start=(n == 0), stop=(n == 3 * NPAIR - 1),
                                perf_mode=DR)
                            n += 1
                    # single eviction: rows 0:64 = K dims (rope input),
                    # rows 64:128 = V dims (read later by the transposes)
                    kvsb = spool.tile([P, 512], BF16, tag="kvsb", bufs=4,
                                      name=f"kvsb_{b}_{half}")
                    nc.vector.tensor_copy(kvsb[:], pj[:])
                    vstage.append(kvsb)
                    rope_half(pj, kh, 64, half, sign_dup=True, sb=kvsb)

                def q_chunk(pairi, half):
                    pj = psm.tile([P, 512], F32, tag="pj",
                                  name=f"qpj_{b}_{pairi}_{half}")
                    cols = (half * 512, (half + 1) * 512)
                    n = 0
                    for wt, xt in ((t_wqh, xts_h), (t_wqr, xts_h),
                                   (t_wqh, xts_r)):
                        for pi in range(NPAIR):
                            nc.tensor.matmul(
                                pj[:],
                                wt[:, 2 * pi:2 * pi + 2,
                                   pairi * P:(pairi + 1) * P],
                                xsrc(xt, pi, cols),
                                start=(n == 0), stop=(n == 3 * NPAIR - 1),
                                perf_mode=DR)
                            n += 1
                    rope_half(pj, qh[pairi], 128, half, sign_dup=False)

                kv_chunk(0)
                q_chunk(0, 0)
                kv_chunk(1)
                q_chunk(0, 1)
                q_chunk(1, 0)
                q_chunk(1, 1)

                # |k|^2 -> per-partition bias  (transposed via PE)
                t_o64, t_idb = lc[1], lc[4]
                nc.scalar.activation(k2[:], kh[0:64, :], AF.Square)
                nsq = psm.tile([P, 512], F32, tag="pj", name=f"nsq_{b}")
                for kc in range(NTC):
                    nc.tensor.matmul(
                        nsq[:, 2 * kc:2 * kc + 2], k2[:, kc * P:(kc + 1) * P],
                        t_o64[:], start=True, stop=True)
                kb = qpool.tile([P, NTC], F32, tag="kb", name=f"kb_{b}")
                nc.vector.tensor_scalar_mul(
                    kb[:],
                    nsq[:, 0:2 * NTC]
                    .rearrange("p (c two) -> p c two", two=2)[:, :, 0],
                    -0.0625)

                # V transposed into [token, hd | c] layout via PE transpose;
                # the ones-column carries c = sx*sw/sy for free dequant
                vh = vpool.tile([P, NTC, HD + 1], BF16, tag="vh",
                                name=f"vh_{b}")
                nc.vector.memset(vh[:, :, HD:HD + 1], ONES_C)
                for tcn in range(NTC):
                    tp = pss.tile([P, 64], BF16, tag="stp", name=f"tp_{b}_{tcn}")
                    vsrc = vstage[tcn // 4]
                    nc.tensor.transpose(
                        tp[:], vsrc[64:128, (tcn % 4) * P:(tcn % 4 + 1) * P],
                        t_idb[64:128, :])
                    nc.vector.tensor_copy(vh[:, tcn, 0:HD], tp[:])
                return dict(b=b, qh=qh, kh=kh, kb=kb, vh=vh)

            def attn_stage(st, filler=None):
                """filler: iterator of thunks (prev batch's out-proj token
                blocks) drained between heads so the out-projection never
                runs as a bare tail with idle engines."""
                b, qh, kh, kb, vh = st["b"], st["qh"], st["kh"], st["kb"], st["vh"]
                t_tri2 = lc[0]
                ytnh = ypool.tile([P, 2, T], FP8, tag="ytnh", bufs=3,
                                  name=f"ytnh_{b}")
                ytnr = ypool.tile([P, 2, T], FP8, tag="ytnr", bufs=3,
                                  name=f"ytnr_{b}")

                def normalize_half(yh_half, hq, pairi, bp, tag):
                    """One half of softmax-normalize as soon as its PV
                    contributions are complete; frees the yh slot early.
                    Emits fp8 hi/res planes of y*sy."""
                    rsb = npool.tile([1, 512], F32R, tag="rsb")
                    with nc.allow_low_precision(reason="recip row"):
                        nc.vector.reciprocal(rsb[:], yh_half[64:65, :])
                    rbc = npool.tile([64, 512], F32R, tag="rbc")
                    nc.gpsimd.partition_broadcast(rbc[:], rsb[:])
                    tmpf = npool.tile([P, 512], F32, tag="ytmp", bufs=2)
                    tmp = tmpf[bp:bp + 64, :]
                    nc.vector.tensor_mul(tmp, yh_half[0:64, :], rbc[:])
                    dsth = ytnh[bp:bp + 64, pairi, hq * 512:(hq + 1) * 512]
                    nc.vector.tensor_copy(dsth, tmp)
                    nc.gpsimd.tensor_sub(
                        ytnr[bp:bp + 64, pairi, hq * 512:(hq + 1) * 512],
                        tmp, dsth)

                for h in range(HPC):
                    pairi, bp = h // 2, 64 * (h % 2)
                    yhA = psy.tile([65, 512], F32, tag="yh",
                                   name=f"yhA_{b}_{h}")
                    yhB = psy.tile([65, 512], F32, tag="yh",
                                   name=f"yhB_{b}_{h}")
                    for kc in range(NTC):
                        q0 = kc * P
                        _, chs = _chunks_for(kc)
                        pt = fpool.tile([P, T], BF16, tag="pt", bufs=6)
                        for (c0, c1) in chs:
                            stp = pss.tile([P, 512], F32, tag="stp")
                            nc.tensor.matmul(
                                stp[:, 0:c1 - c0],
                                kh[bp:bp + 64, kc * P:(kc + 1) * P],
                                qh[pairi][bp:bp + 64, c0:c1],
                                start=True, stop=True,
                                tile_position=(bp, 0))
                            nc.scalar.activation(
                                pt[:, c0:c1], stp[:, 0:c1 - c0], AF.Exp,
                                bias=kb[:, kc:kc + 1], scale=0.125)
                        # causal mask on the diagonal block (SBUF-only: gpsimd)
                        nc.gpsimd.tensor_mul(
                            pt[:, q0:q0 + P], pt[:, q0:q0 + P],
                            t_tri2[:, 0, :])
                        # split PV at the diagonal boundary so the
                        # unmasked bulk starts right after exp (no mask hop)
                        pvs = []
                        for (c0, c1) in chs:
                            if c0 == q0 and c1 > q0 + P:
                                pvs += [(c0, q0 + P), (q0 + P, c1)]
                            else:
                                pvs.append((c0, c1))
                        for (c0, c1) in pvs:
                            half = yhA if c0 < 512 else yhB
                            off = 0 if c0 < 512 else 512
                            nc.tensor.matmul(
                                half[:, c0 - off:c1 - off], vh[:, kc, :],
                                pt[:, c0:c1],
                                start=(kc == 0),
                                stop=(kc == (3 if half is yhA else NTC - 1)),
                                skip_group_check=True)
                        if kc == 3:
                            normalize_half(yhA, 0, pairi, bp, "A")
                    normalize_half(yhB, 1, pairi, bp, "B")
                    if filler is not None:
                        for _ in range(2):
                            thunk = next(filler, None)
                            if thunk is not None:
                                thunk()

                return ytnh, ytnr

            # ostg evictions alternate DVE/Act (gpsimd cannot read PSUM)
            _evict_rr = [nc.vector, nc.scalar]

            def out_tcn(b, ytn, tcn):
                """One token-block of the output projection (a filler unit)."""
                ytnh, ytnr = ytn
                tok0 = b * T
                t_woh, t_wor = lc[2], lc[3]
                ostg = opool.tile([P, D], BF16, tag="ostg", bufs=2)
                for oc in range(4):
                    ops_ = pss.tile([P, 512], F32, tag="stp",
                                    name=f"ops_{b}_{tcn}_{oc}")
                    terms = ((ytnh, t_woh), (ytnr, t_woh), (ytnh, t_wor))
                    for n, (yt, wt) in enumerate(terms):
                        nc.tensor.matmul(
                            ops_[:],
                            yt[:, :, tcn * P:(tcn + 1) * P],
                            wt[:, :, oc * 512:(oc + 1) * 512],
                            start=(n == 0), stop=(n == 2),
                            perf_mode=DR)
                    eng = _evict_rr[(tcn * 4 + oc) % 2]
                    if eng is nc.scalar:
                        nc.scalar.activation(
                            ostg[:, oc * 512:(oc + 1) * 512], ops_[:],
                            AF.Copy, scale=OUT_SC)
                    else:
                        eng.tensor_scalar_mul(
                            ostg[:, oc * 512:(oc + 1) * 512], ops_[:], OUT_SC)
                if b == B - 1 and tcn >= NTC - 2:
                    # pipeline the final drain: two half-width DMAs so
                    # the first launches while oc2/oc3 still evict
                    for hh in range(2):
                        nc.sync.dma_start(
                            out.ap()[tok0 + tcn * P: tok0 + (tcn + 1) * P,
                                     hh * 1024:(hh + 1) * 1024],
                            ostg[:, hh * 1024:(hh + 1) * 1024])
                else:
                    nc.sync.dma_start(
                        out.ap()[tok0 + tcn * P: tok0 + (tcn + 1) * P, :],
                        ostg[:])

            def out_stage(b, ytn):
                for tcn in range(NTC):
                    out_tcn(b, ytn, tcn)

            # software pipeline with deferred out-projections: proj/rope
            # of b+1 runs ahead of attention of b, and outproj of b is
            # issued a full stage later so it stays available as PE filler
            # for the Act-rate-limited late attention phases. The last two
            # out-projections interleave into attention 3 per-head so the
            # eviction engines are never a bare serial tail.
            # Order: P0 P1 A0 P2 A1 O0 P3 A2 O1 A3(+O2) O3
            st0 = proj_rope_stage(0)
            st1 = proj_rope_stage(1)
            y0 = attn_stage(st0)
            st2 = proj_rope_stage(2)
            y1 = attn_stage(st1)
            out_stage(0, y0)
            st3 = proj_rope_stage(3)
            y2 = attn_stage(st2)
            out_stage(1, y1)
            o2 = iter([(lambda b_, y_, t_: (lambda: out_tcn(b_, y_, t_)))(2, y2, t)
                       for t in range(NTC)])
            y3 = attn_stage(st3, filler=o2)
            for thunk in o2:
                thunk()
            out_stage(3, y3)

    nc.finalize()
    return nc


def _host_consts():
    inv = 1.0 / (ROPE_BASE ** (np.arange(0, HD, 2, dtype=np.float32) / HD))
    ang = np.arange(T, dtype=np.float32)[:, None] * inv[None, :]  # [T, 32]
    dq = 1.0 / (SX * SW)
    cosr = (np.cos(ang) * dq).T.astype(np.float32)                 # [32, T]
    sinr = (np.sin(ang) * dq).T.astype(np.float32)
    cc = np.tile(cosr, (4, 1))                                     # [128, T]
    ss = np.tile(np.concatenate([sinr, -sinr], axis=0), (2, 1))
    tri = np.triu(np.ones((P, P), np.float32))
    tri2 = np.stack([tri, tri], axis=1)                            # [128,2,128]
    return {
        "cc": np.ascontiguousarray(cc.astype(BF)),
        "ss": np.ascontiguousarray(ss.astype(BF)),
        "tri2": np.ascontiguousarray(tri2.astype(BF)),
        "o64": np.ones((64, 2), np.float32),
        "idb": np.ascontiguousarray(np.concatenate(
            [np.zeros((64, 64), np.float32),
             np.eye(64, dtype=np.float32)], axis=0).astype(BF)),
    }


def _split8(a, scale):
    """a*scale ~= hi + res, both fp8 bytes."""
    hi = np.clip(a * scale, -240.0, 240.0).astype(F8)
    res = (a * scale - hi.astype(np.float32)).astype(F8)
    return hi, res


def kernel(x, Wq, Wk, Wv, Wo):
    x = np.asarray(x, np.float32)
    Wq = np.asarray(Wq, np.float32)
    Wk = np.asarray(Wk, np.float32)
    Wv = np.asarray(Wv, np.float32)
    Wo = np.asarray(Wo, np.float32)
    b, t, d = x.shape

    key = "nc"
    if key not in _COMPILED:
        _COMPILED[key] = _build_nc()
    nc = _COMPILED[key]

    xT = x.reshape(b * t, d).T                                   # [2048, 4096]
    xh_np, xr_np = _split8(np.ascontiguousarray(xT), SX)
    consts = _host_consts()

    in_maps = []
    for c in range(NCORES):
        wq_c = (Wq[:, c * DOUT:(c + 1) * DOUT] * SW).reshape(NDC, P, DOUT) \
            .transpose(1, 0, 2)
        wqh_c = np.clip(wq_c, -240, 240).astype(F8)
        wqr_c = (wq_c - wqh_c.astype(np.float32)).astype(F8)
        wkv_np = np.concatenate(
            [Wk[:, c * HD:(c + 1) * HD], Wv[:, c * HD:(c + 1) * HD]],
            axis=1) * SW
        wkv_c = wkv_np.reshape(NDC, P, 2 * HD).transpose(1, 0, 2)
        wkvh_c = np.clip(wkv_c, -240, 240).astype(F8)
        wkvr_c = (wkv_c - wkvh_c.astype(np.float32)).astype(F8)
        wo_c = (Wo[c * DOUT:(c + 1) * DOUT, :] * SWO).reshape(2, P, d) \
            .transpose(1, 0, 2)
        woh_c = np.clip(wo_c, -240, 240).astype(F8)
        wor_c = (wo_c - woh_c.astype(np.float32)).astype(F8)
        m = {"xh": xh_np, "xr": xr_np,
             "wqh": np.ascontiguousarray(wqh_c),
             "wqr": np.ascontiguousarray(wqr_c),
             "wkvh": np.ascontiguousarray(wkvh_c),
             "wkvr": np.ascontiguousarray(wkvr_c),
             "woh": np.ascontiguousarray(woh_c),
             "wor": np.ascontiguousarray(wor_c)}
        m.update(consts)
        in_maps.append(m)

    res = run_bass_kernel_spmd(nc, in_maps, list(range(NCORES)))
    acc = res.results[0]["out"].astype(np.float32)
    for c in range(1, NCORES):
        acc = acc + res.results[c]["out"].astype(np.float32)
    return acc.reshape(b, t, d)


if __name__ == "__main__":
    rng = np.random.default_rng(0)
    x = rng.standard_normal((B, T, D), dtype=np.float32)
    Wq = (rng.standard_normal((D, D), dtype=np.float32) * 0.02)
    Wk = (rng.standard_normal((D, KV), dtype=np.float32) * 0.02)
    Wv = (rng.standard_normal((D, KV), dtype=np.float32) * 0.02)
    Wo = (rng.standard_normal((D, D), dtype=np.float32) * 0.02)
    y = kernel(x=x, Wq=Wq, Wk=Wk, Wv=Wv, Wo=Wo)
    print("out", y.shape, y.dtype, np.abs(y).max())
